# revision 1
# baseline (speedup 1.0000x reference)
"""nn_ChannelAttention Trainium2 Bass kernel (8-core SPMD, data-parallel over batch).

Input  x:   [8, 64, 32, 128, 128] f32
Output att: [8, 64, 1, 1, 1] f32
  n[s]   = sum_c x[c,s]^2
  r[s]   = 1/sqrt(n[s]) = exp(-0.5*ln n)   (channel norms are ~8, eps clamp is a no-op)
  att[c] = sigmoid( relu( mean_s(x*r) + max_s(x*r) )^2 )

Per-core layout: SBUF partitions = (h, c) with h in {0,1} spatial halves,
free = spatial. Tiles [128, F]; 4-tile batches share one PSUM buffer:
  - channel-sumsq via PE matmul (block [128,32] ones lhsT -> 32-row band per
    tile at bases {0,32,64,96}; 16x redundant but PE cost is N-cycle, M-free)
  - ACT Ln then Exp(scale=-0.5) on the redundant [128,F] buffer -> rsqrt
  - PE matmul broadcasts each tile's [2,F] r-slice to [128,F] PSUM; ACT Copy
    exits it to SBUF fp16
  - DVE tensor_mul + tensor_max chains; spatial sum via PE identity-matmul
    accumulation into a resident PSUM tile
"""

from contextlib import ExitStack

import numpy as np

import concourse.bass as bass
import concourse.mybir as mybir
import concourse.tile as tile

F32 = mybir.dt.float32
FP16 = mybir.dt.float16
AF = mybir.ActivationFunctionType
ALU = mybir.AluOpType

B, C, D, H, W = 8, 64, 32, 128, 128
S = D * H * W            # 524288 spatial positions per sample
N_CORES = 8

# tuning knobs
F_TILE = 512             # tile free width (smaller ops pay less DVE drain and
                         # allow nacc/rbp double-buffering within 8 PSUM banks)
TPG = 16                 # tiles per DMA group
SQ_ACT_MOD = 5           # every k-th square on ACT, rest on DVE (cast_dma path)
EXIT_PSUM_MOD = 0        # every k-th tile multiplies straight out of PSUM
CAST_DMA = False         # SWDGE f32->fp16 cast DMA measured ~10x below line
                         # rate; load f32 and keep x in f32 instead


def _build_kernel_body(nc, F=F_TILE, TPG=TPG, comp_dt=FP16, sq_act_mod=SQ_ACT_MOD,
                       dma_split=1, exit_psum_mod=EXIT_PSUM_MOD, repeat=1,
                       cast_dma=CAST_DMA):
    C_, P = 64, 128
    HALF = S // 2
    MM = 512                       # one PSUM bank of f32
    NT = (S * C_) // (P * F)       # total tiles
    GF = F * TPG
    NG = NT // TPG
    assert NG * GF == HALF and TPG % 4 == 0

    x = nc.dram_tensor("x", [C_, S], F32, kind="ExternalInput")
    y = nc.dram_tensor("att", [C_, 1], F32, kind="ExternalOutput")
    xr = x.ap().rearrange("c (h s) -> h c s", h=2)   # element order (h, c, s)

    with tile.TileContext(nc) as tc, ExitStack() as ctx:
        const_pool = ctx.enter_context(tc.tile_pool(name="const", bufs=1))
        gbuf_pool = ctx.enter_context(tc.tile_pool(name="gbuf", bufs=2))
        sq_pool = ctx.enter_context(tc.tile_pool(name="sq", bufs=8))
        nacc_pool = ctx.enter_context(tc.tile_pool(
            name="nacc", bufs=(2 if F <= 512 else 1), space="PSUM"))
        sacc_pool = ctx.enter_context(tc.tile_pool(name="sacc", bufs=1, space="PSUM"))
        rbp_pool = ctx.enter_context(tc.tile_pool(
            name="rbp", bufs=(4 if F <= 512 else 2), space="PSUM"))
        rb_pool = ctx.enter_context(tc.tile_pool(name="rb", bufs=6))
        xn_pool = ctx.enter_context(tc.tile_pool(name="xn", bufs=6))
        lall_pool = ctx.enter_context(tc.tile_pool(name="lall", bufs=3))
        acc_pool = ctx.enter_context(tc.tile_pool(name="acc", bufs=1))
        fin_pool = ctx.enter_context(tc.tile_pool(name="fin", bufs=1))

        # lhsT32[p, m] = 1 iff p//64 == m%2  (out row m = n[h=m%2])
        lhsT32 = const_pool.tile([P, 32], comp_dt)
        nc.vector.memset(lhsT32[:], 0.0)
        lo = lhsT32[0:64, :].rearrange("p (m two) -> p m two", two=2)
        nc.vector.memset(lo[:, :, 0:1], 1.0)
        hi = lhsT32[64:128, :].rearrange("p (m two) -> p m two", two=2)
        nc.vector.memset(hi[:, :, 1:2], 1.0)
        # sel2_all rows 32b+h: ones at cols h*64:(h+1)*64 (placed by DMA --
        # engine ops cannot start at partition 32b+1)
        sel2_all = const_pool.tile([P, P], comp_dt)
        nc.vector.memset(sel2_all[:], 0.0)
        rowpat = const_pool.tile([1, 2 * P], comp_dt)
        nc.vector.memset(rowpat[:], 0.0)
        nc.vector.memset(rowpat[0:1, 0:64], 1.0)
        nc.vector.memset(rowpat[0:1, 192:256], 1.0)
        for b4 in range(4):
            nc.gpsimd.dma_start(sel2_all[32 * b4:32 * b4 + 2, :], rowpat[0:1, :])

        ident = const_pool.tile([P, P], comp_dt)
        ones_t = const_pool.tile([P, P], comp_dt)
        nc.vector.memset(ones_t[:], 1.0)
        nc.gpsimd.affine_select(ident[:], ones_t[:], pattern=[[1, P]], base=0,
                                channel_multiplier=-1, compare_op=ALU.is_equal,
                                fill=0.0)

        sacc = sacc_pool.tile([P, F], F32)
        macc_a0 = acc_pool.tile([P, F], comp_dt)
        macc_b0 = acc_pool.tile([P, F], comp_dt)
        macc_a1 = acc_pool.tile([P, F], comp_dt)
        macc_b1 = acc_pool.tile([P, F], comp_dt)
        maccs = [[macc_a0, macc_b0], [macc_a1, macc_b1]]
        for pair in maccs:
            nc.vector.memset(pair[0][:], -2.0)
            nc.vector.memset(pair[1][:], -2.0)

        sq_ct = 0
        for rep in range(repeat):         # >1 only for timing builds
            for g in range(NG):
                gbuf = gbuf_pool.tile([P, F * TPG], comp_dt if cast_dma else F32)
                step = GF // dma_split
                for d in range(dma_split):
                    nc.gpsimd.dma_start(
                        gbuf[:, d * step:(d + 1) * step],
                        xr[:, :, g * GF + d * step: g * GF + (d + 1) * step])

                for bb in range(TPG // 4):
                    nacc = nacc_pool.tile([P, F], F32)
                    for b in range(4):
                        t = bb * 4 + b
                        x_t = gbuf[:, t * F:(t + 1) * F]
                        sq = sq_pool.tile([P, F], comp_dt, tag="sq")
                        sq_ct += 1
                        if (not cast_dma) or sq_act_mod == 0 or (sq_ct % sq_act_mod == 0):
                            # ACT is 1x for all dtypes, so f32 input is free here
                            nc.scalar.square(sq[:], x_t)
                        else:
                            nc.vector.tensor_mul(sq[:], x_t, x_t)
                        for m0 in range(0, F, MM):
                            nc.tensor.matmul(
                                nacc[32 * b:32 * b + 32, m0:m0 + MM],
                                lhsT32[:], sq[:, m0:m0 + MM],
                                start=True, stop=True, tile_position=(0, 32 * b))

                    l_all = lall_pool.tile([P, F], F32, tag="lall")
                    nc.scalar.activation(l_all[:], nacc[:], AF.Ln)
                    r_all = lall_pool.tile([P, F], comp_dt, tag="rall")
                    nc.scalar.activation(r_all[:], l_all[:], AF.Exp, scale=-0.5)

                    for b in range(4):
                        t = bb * 4 + b
                        gi = g * TPG + t
                        gl = rep * NT + gi
                        x_t = gbuf[:, t * F:(t + 1) * F]
                        rbp = rbp_pool.tile([P, F], F32, tag="rbp")
                        for m0 in range(0, F, MM):
                            nc.tensor.matmul(
                                rbp[:, m0:m0 + MM],
                                sel2_all[32 * b:32 * b + 2, :],
                                r_all[32 * b:32 * b + 2, m0:m0 + MM],
                                start=True, stop=True, tile_position=(32 * b, 0))

                        xn1 = xn_pool.tile([P, F], comp_dt, tag="xn")
                        if not cast_dma:
                            # f32 x forces 1x mode anyway, so read rbp straight
                            # from PSUM and skip the ACT exit pass entirely
                            nc.vector.tensor_mul(xn1[:], x_t, rbp[:])
                        elif exit_psum_mod and (gi % exit_psum_mod) == 0:
                            nc.vector.tensor_mul(xn1[:], x_t, rbp[:])
                        else:
                            rb = rb_pool.tile([P, F], comp_dt, tag="rb")
                            nc.scalar.activation(rb[:], rbp[:], AF.Copy)
                            nc.vector.tensor_mul(xn1[:], x_t, rb[:])
                        for m0 in range(0, F, MM):
                            nc.tensor.matmul(
                                sacc[:, m0:m0 + MM], ident[:], xn1[:, m0:m0 + MM],
                                start=(gl == 0), stop=(gl == repeat * NT - 1),
                                skip_group_check=True)
                        pair = maccs[(gi // 2) % 2]
                        src, dst = pair[gi % 2], pair[1 - gi % 2]
                        nc.vector.tensor_max(dst[:], src[:], xn1[:])

        # ---- finalize ----
        sum_pc = fin_pool.tile([P, 1], F32)
        s_sb = fin_pool.tile([P, F], F32)
        nc.scalar.activation(s_sb[:], sacc[:], AF.Copy)
        nc.vector.reduce_sum(sum_pc[:], s_sb[:], axis=mybir.AxisListType.X)
        mfin0 = fin_pool.tile([P, F], comp_dt)
        nc.vector.tensor_max(mfin0[:], maccs[0][0][:], maccs[0][1][:])
        mfin1 = fin_pool.tile([P, F], comp_dt)
        nc.vector.tensor_max(mfin1[:], maccs[1][0][:], maccs[1][1][:])
        mfin = fin_pool.tile([P, F], comp_dt)
        nc.vector.tensor_max(mfin[:], mfin0[:], mfin1[:])
        max_pc = fin_pool.tile([P, 1], F32)
        nc.vector.reduce_max(max_pc[:], mfin[:], axis=mybir.AxisListType.X)

        # fold halves (partitions 64:128 -> 0:64) via SBUF->SBUF DMA realign
        hi2 = fin_pool.tile([64, 2], F32)
        nc.gpsimd.dma_start(hi2[:, 0:1], sum_pc[64:128, :])
        nc.gpsimd.dma_start(hi2[:, 1:2], max_pc[64:128, :])
        s64 = fin_pool.tile([64, 1], F32)
        nc.vector.tensor_add(s64[:], sum_pc[0:64, :], hi2[:, 0:1])
        m64 = fin_pool.tile([64, 1], F32)
        nc.vector.tensor_max(m64[:], max_pc[0:64, :], hi2[:, 1:2])
        avg = fin_pool.tile([64, 1], F32)
        nc.vector.tensor_scalar_mul(avg[:], s64[:], 1.0 / (S * repeat))
        o = fin_pool.tile([64, 1], F32)
        nc.vector.tensor_add(o[:], avg[:], m64[:])
        orelu = fin_pool.tile([64, 1], F32)
        nc.vector.tensor_scalar_max(orelu[:], o[:], 0.0)
        o2 = fin_pool.tile([64, 1], F32)
        nc.vector.tensor_mul(o2[:], orelu[:], orelu[:])
        att_s = fin_pool.tile([64, 1], F32)
        nc.scalar.activation(att_s[:], o2[:], AF.Sigmoid)
        nc.gpsimd.dma_start(y.ap(), att_s[:])
    return nc


def _split_multi_waits(nc, max_waits=1):
    """This walrus build encodes at most one sync-wait per CTRL instruction;
    hoist extra waits into single-wait NoOps placed just before."""
    for f in nc.m.functions:
        for bb in f.blocks:
            insts = list(bb.instructions)
            out = []
            changed = False
            for ins in insts:
                si = ins.sync_info
                if si is not None and si.on_wait and len(si.on_wait) > max_waits:
                    waits = list(si.on_wait)
                    for w in waits[:-max_waits]:
                        out.append(mybir.InstNoOp(
                            name=nc.get_next_instruction_name(),
                            sync_info=mybir.SyncInfo(on_wait=[w], on_update=[]),
                            bass_nofuse=True,
                            engine=ins.engine,
                        ))
                    si.on_wait = waits[-max_waits:]
                    ins.sync_info = si
                    changed = True
                out.append(ins)
            if changed:
                bb.instructions = out


def build_nc(repeat=1, **kw):
    nc = bass.Bass("TRN2", target_bir_lowering=False, debug=False,
                   num_devices=N_CORES)
    _build_kernel_body(nc, repeat=repeat, **kw)
    _split_multi_waits(nc)
    return nc


def kernel(x):
    """x: [8, 64, 32, 128, 128] f32 -> att [8, 64, 1, 1, 1] f32."""
    from concourse.bass_utils import run_bass_kernel_spmd

    x = np.ascontiguousarray(np.asarray(x, dtype=np.float32))
    assert x.shape == (B, C, D, H, W)
    nc = build_nc()
    in_maps = [{"x": x[i].reshape(C, S)} for i in range(N_CORES)]
    res = run_bass_kernel_spmd(nc, in_maps, core_ids=list(range(N_CORES)))
    att = np.stack([res.results[i]["att"].reshape(C) for i in range(N_CORES)])
    return att.reshape(B, C, 1, 1, 1).astype(np.float32)



# revision 9
# speedup vs baseline: 1.0053x; 1.0053x over previous
"""nn_ChannelAttention Trainium2 Bass kernel v2 (8-core SPMD, data-parallel).

Input  x:   [8, 64, 32, 128, 128] f32
Output att: [8, 64, 1, 1, 1] f32
  n[s]   = sum_c x[c,s]^2
  r[s]   = rsqrt(n[s])        (channel norms ~8, the eps clamp is a no-op)
  att[c] = sigmoid( relu( mean_s(x*r) + max_s(x*r) )^2 )

Per-core layout: SBUF partitions = (h, c) with h in {0,1} spatial halves,
free = spatial. Tiles [128, F]; 4-tile batches share one PSUM buffer:
  - one fat ACT Square per 4-tile batch ([128, 4F]); PE band matmuls with a
    ones lhsT produce per-position channel sumsq; ACT Ln + Exp(-0.5*) -> r
    (ln/exp/square/copy share one activation table -> no table reloads)
  - PE matmul broadcasts each tile's [2,F] r-slice to [128,F] PSUM
  - one custom DVE op (TTR_MAX_ANT, authored via the dve_ops extension
    point) per tile computes xn = x*r AND writes max_s(xn) into its own
    column of a wide [128, NT] accumulator; final reduce_max folds it.
    (The native InstTensorTensorReduce op1=max hangs the DVE on HW.)
  - spatial sum via PE identity-matmul accumulation into resident PSUM
  - group loads split between the HWDGE(SP) and SWDGE(Pool) DMA queues
    ("mix2"): a single queue tops out at ~170 GB/s/core on HW; two queues
    overlap transfers and recover most of the gap
"""

from contextlib import ExitStack

import numpy as np

import concourse.bass as bass
import concourse.mybir as mybir
import concourse.tile as tile

F32 = mybir.dt.float32
FP16 = mybir.dt.float16
AF = mybir.ActivationFunctionType
ALU = mybir.AluOpType

B, C, D, H, W = 8, 64, 32, 128, 128
S = D * H * W            # 524288 spatial positions per sample
N_CORES = 8


def _ttr_max_op():
    """Register (idempotently) a custom DVE op via the documented dve_ops
    extension point:  out = in0*in1*c1 ; accum_out = max(c0, rowmax(out)).

    The native InstTensorTensorReduce encodes op1=max but the HW uop table
    only implements add-accum (and the encoding hangs the DVE -> mesh
    desync), so we author the max-accum variant properly.
    """
    from concourse import dve_ops
    from concourse.dve_spec import Spec, Src0, Src1, C0, C1, lower, maxx
    from concourse.dve_spec import _has_src1 as has_src1
    from concourse.dve_uop import DveOpSpec

    name = "TTR_MAX_ANT"
    for op in dve_ops.OPS:
        if op.name == name:
            return op

    def _ref(in0, in1, c0, c1, c2):
        b = (in0.astype(np.float32) * in1 * c1).astype(np.float32)
        mx = np.maximum(c0, b.reshape(b.shape[0], -1).max(axis=-1, keepdims=True))
        return b, mx

    spec = Spec(body=Src0 * Src1 * C1, accum=maxx, accum_init=C0,
                reference=_ref)
    row = dve_ops._CUSTOM_DVE_ROW_BASE + len(dve_ops.OPS)
    assert row < 0x20, "no free custom-DVE opcode rows"
    shas = {}
    for ver in ("v3", "v4"):
        s = DveOpSpec(name=name, opcode=row, uops=lower(spec, ver=ver),
                      rd1_en=has_src1(spec))
        shas[ver] = s.sha(ver)
    op = dve_ops.DveOp(name, spec, subdim=False, uops_sha=shas)
    dve_ops.OPS.append(op)
    dve_ops.CUSTOM_DVE_SPECS[name] = spec
    dve_ops._SUB_OPCODE_FOR_NAME[name] = row
    return op

F_TILE = 512             # tile free width
TPG = 16                 # tiles per DMA group
N_ROT = 4                # rotating DVE max accumulators


def _build_kernel_body(nc, F=F_TILE, TPG=TPG, repeat=1, mode="full",
                       dma_eng="mix2", gbuf_bufs=3, nacc_bufs=2, rbp_bufs=4,
                       fuse=True, rsq="lnexp", sq_span=4):
    C_, P = 64, 128
    HALF = S // 2
    MM = 512                       # one PSUM bank of f32
    NT = (S * C_) // (P * F)       # total tiles
    GF = F * TPG
    NG = NT // TPG
    assert NG * GF == HALF and TPG % 4 == 0

    x = nc.dram_tensor("x", [C_, S], F32, kind="ExternalInput")
    y = nc.dram_tensor("att", [C_, 1], F32, kind="ExternalOutput")
    xr = x.ap().rearrange("c (h s) -> h c s", h=2)   # element order (h, c, s)
    _TTRMAX = _ttr_max_op() if fuse else None

    with tile.TileContext(nc) as tc, ExitStack() as ctx:
        const_pool = ctx.enter_context(tc.tile_pool(name="const", bufs=1))
        gbuf_pool = ctx.enter_context(tc.tile_pool(name="gbuf", bufs=gbuf_bufs))
        sq_pool = ctx.enter_context(
            tc.tile_pool(name="sq", bufs=(8 if sq_span == 1 else 2)))
        nacc_pool = ctx.enter_context(tc.tile_pool(
            name="nacc", bufs=nacc_bufs, space="PSUM"))
        sacc_pool = ctx.enter_context(tc.tile_pool(name="sacc", bufs=1, space="PSUM"))
        rbp_pool = ctx.enter_context(tc.tile_pool(
            name="rbp", bufs=rbp_bufs, space="PSUM"))
        rall_pool = ctx.enter_context(tc.tile_pool(name="rall", bufs=3))
        xn_pool = ctx.enter_context(tc.tile_pool(name="xn", bufs=6))
        acc_pool = ctx.enter_context(tc.tile_pool(name="acc", bufs=1))
        fin_pool = ctx.enter_context(tc.tile_pool(name="fin", bufs=1))

        # lhsT32[p, m] = 1 iff p//64 == m%2  (out row m = n[h=m%2])
        lhsT32 = const_pool.tile([P, 32], FP16)
        nc.vector.memset(lhsT32[:], 0.0)
        lo = lhsT32[0:64, :].rearrange("p (m two) -> p m two", two=2)
        nc.vector.memset(lo[:, :, 0:1], 1.0)
        hi = lhsT32[64:128, :].rearrange("p (m two) -> p m two", two=2)
        nc.vector.memset(hi[:, :, 1:2], 1.0)
        # sel2_all rows 32b+h: ones at cols h*64:(h+1)*64 (placed by DMA --
        # engine ops cannot start at partition 32b+1)
        sel2_all = const_pool.tile([P, P], FP16)
        nc.vector.memset(sel2_all[:], 0.0)
        rowpat = const_pool.tile([1, 2 * P], FP16)
        nc.vector.memset(rowpat[:], 0.0)
        nc.vector.memset(rowpat[0:1, 0:64], 1.0)
        nc.vector.memset(rowpat[0:1, 192:256], 1.0)
        for b4 in range(4):
            nc.gpsimd.dma_start(sel2_all[32 * b4:32 * b4 + 2, :], rowpat[0:1, :])

        ident = const_pool.tile([P, P], FP16)
        ones_t = const_pool.tile([P, P], FP16)
        nc.vector.memset(ones_t[:], 1.0)
        nc.gpsimd.affine_select(ident[:], ones_t[:], pattern=[[1, P]], base=0,
                                channel_multiplier=-1, compare_op=ALU.is_equal,
                                fill=0.0)

        sacc = sacc_pool.tile([P, MM], F32)
        NCOL = repeat * NT if mode != "compute" else repeat * NT
        macc_w = acc_pool.tile([P, NCOL], F32)
        # fallback (fuse=False) full-tile max ping-pong buffers
        if not fuse:
            mp_a = acc_pool.tile([P, F], FP16)
            mp_b = acc_pool.tile([P, F], FP16)
            nc.vector.memset(mp_a[:], -2.0)
            nc.vector.memset(mp_b[:], -2.0)

        # weights balance the two DGE paths: SWDGE (gpsimd) sustains ~3x the
        # per-queue rate of HWDGE (sync) on HW, so give it 3/4 of each group
        dma_engs = {"gpsimd": [(nc.gpsimd, 1)], "sync": [(nc.sync, 1)],
                    "scalar": [(nc.scalar, 1)],
                    "mix2": [(nc.sync, 1), (nc.gpsimd, 1)],
                    "mix2w": [(nc.sync, 1), (nc.gpsimd, 3)],
                    "mix3": [(nc.sync, 1), (nc.scalar, 1), (nc.gpsimd, 1)]}
        des = dma_engs[dma_eng]
        if mode == "compute":
            gbuf0 = gbuf_pool.tile([P, GF], F32)
            nc.gpsimd.dma_start(gbuf0[:], xr[:, :, 0:GF])

        for rep in range(repeat):         # >1 only for timing builds
            for g in range(NG):
                if mode == "compute":
                    gbuf = gbuf0
                else:
                    gbuf = gbuf_pool.tile([P, GF], F32)
                    if len(des) == 1:
                        des[0][0].dma_start(gbuf[:],
                                            xr[:, :, g * GF:(g + 1) * GF])
                    else:
                        # split each group across engines -> parallel queues
                        wtot = sum(w for _, w in des)
                        unit = GF // wtot
                        assert unit * wtot == GF
                        o = 0
                        for eng, w in des:
                            sz = unit * w
                            eng.dma_start(
                                gbuf[:, o:o + sz],
                                xr[:, :, g * GF + o:g * GF + o + sz])
                            o += sz
                if mode == "dma":
                    # tiny consumer: forces a completion sem per group DMA so
                    # the final barrier actually waits for the transfers
                    dprobe = fin_pool.tile([1, 1], F32, tag="dprobe", bufs=2)
                    nc.vector.tensor_copy(dprobe[:], gbuf[0:1, 0:1])
                    continue

                for bb in range(TPG // 4):
                    nacc = nacc_pool.tile([P, F], F32)
                    if sq_span == 4:
                        # one fat ACT square for the whole 4-tile batch
                        sq4 = sq_pool.tile([P, 4 * F], FP16, tag="sq")
                        nc.scalar.square(sq4[:], gbuf[:, bb * 4 * F:(bb + 1) * 4 * F])
                    for b in range(4):
                        t = bb * 4 + b
                        x_t = gbuf[:, t * F:(t + 1) * F]
                        if sq_span == 4:
                            sq = sq4[:, b * F:(b + 1) * F]
                        else:
                            sqt = sq_pool.tile([P, F], FP16, tag="sq")
                            nc.scalar.square(sqt[:], x_t)
                            sq = sqt[:]
                        for m0 in range(0, F, MM):
                            nc.tensor.matmul(
                                nacc[32 * b:32 * b + 32, m0:m0 + MM],
                                lhsT32[:], sq[:, m0:m0 + MM],
                                start=True, stop=True, tile_position=(0, 32 * b))

                    r_all = rall_pool.tile([P, F], FP16, tag="rall")
                    if rsq == "abs_rsqrt":
                        # n = sum of squares > 0, so 1/sqrt(|n|) == 1/sqrt(n)
                        nc.scalar.activation(r_all[:], nacc[:],
                                             AF.Abs_reciprocal_sqrt)
                    else:
                        l_all = rall_pool.tile([P, F], F32, tag="lall")
                        nc.scalar.activation(l_all[:], nacc[:], AF.Ln)
                        nc.scalar.activation(r_all[:], l_all[:], AF.Exp,
                                             scale=-0.5)

                    for b in range(4):
                        t = bb * 4 + b
                        gi = (rep * NT if mode != "compute" else 0) \
                            + g * TPG + t
                        gl = rep * NT + g * TPG + t
                        x_t = gbuf[:, t * F:(t + 1) * F]
                        rbp = rbp_pool.tile([P, F], F32, tag="rbp")
                        for m0 in range(0, F, MM):
                            nc.tensor.matmul(
                                rbp[:, m0:m0 + MM],
                                sel2_all[32 * b:32 * b + 2, :],
                                r_all[32 * b:32 * b + 2, m0:m0 + MM],
                                start=True, stop=True, tile_position=(32 * b, 0))

                        xn1 = xn_pool.tile([P, F], FP16, tag="xn")
                        if fuse:
                            nc.vector._custom_dve(
                                _TTRMAX, out=xn1[:], in0=x_t, in1=rbp[:],
                                s0=-2.0, s1=1.0,
                                accum_out=macc_w[:, gl:gl + 1])
                        else:
                            nc.vector.tensor_mul(xn1[:], x_t, rbp[:])
                            src, dst = (mp_a, mp_b) if gl % 2 == 0 else (mp_b, mp_a)
                            nc.vector.tensor_max(dst[:], src[:], xn1[:])
                        for m0 in range(0, F, MM):
                            nc.tensor.matmul(
                                sacc[:], ident[:], xn1[:, m0:m0 + MM],
                                start=(gl == 0 and m0 == 0),
                                stop=(gl == repeat * NT - 1 and m0 == F - MM),
                                skip_group_check=True)

        # ---- finalize ----
        if mode == "dma":
            zatt = fin_pool.tile([64, 1], F32)
            nc.vector.memset(zatt[:], 0.5)
            nc.gpsimd.dma_start(y.ap(), zatt[:])
            return nc

        sum_pc = fin_pool.tile([P, 1], F32)
        s_sb = fin_pool.tile([P, MM], F32)
        nc.scalar.activation(s_sb[:], sacc[:], AF.Copy)
        nc.vector.reduce_sum(sum_pc[:], s_sb[:], axis=mybir.AxisListType.X)

        max_pc = fin_pool.tile([P, 1], F32)
        if fuse:
            nc.vector.reduce_max(max_pc[:], macc_w[:],
                                 axis=mybir.AxisListType.X)
        else:
            mfin = fin_pool.tile([P, F], FP16)
            nc.vector.tensor_max(mfin[:], mp_a[:], mp_b[:])
            nc.vector.reduce_max(max_pc[:], mfin[:], axis=mybir.AxisListType.X)

        # fold halves (partitions 64:128 -> 0:64) via SBUF->SBUF DMA realign
        hi2 = fin_pool.tile([64, 2], F32)
        nc.gpsimd.dma_start(hi2[:, 0:1], sum_pc[64:128, :])
        nc.gpsimd.dma_start(hi2[:, 1:2], max_pc[64:128, :])
        s64 = fin_pool.tile([64, 1], F32)
        nc.vector.tensor_add(s64[:], sum_pc[0:64, :], hi2[:, 0:1])
        m64 = fin_pool.tile([64, 1], F32)
        nc.vector.tensor_max(m64[:], max_pc[0:64, :], hi2[:, 1:2])
        avg = fin_pool.tile([64, 1], F32)
        nc.vector.tensor_scalar_mul(avg[:], s64[:], 1.0 / (S * repeat))
        o = fin_pool.tile([64, 1], F32)
        nc.vector.tensor_add(o[:], avg[:], m64[:])
        orelu = fin_pool.tile([64, 1], F32)
        nc.vector.tensor_scalar_max(orelu[:], o[:], 0.0)
        o2 = fin_pool.tile([64, 1], F32)
        nc.vector.tensor_mul(o2[:], orelu[:], orelu[:])
        att_s = fin_pool.tile([64, 1], F32)
        nc.scalar.activation(att_s[:], o2[:], AF.Sigmoid)
        nc.gpsimd.dma_start(y.ap(), att_s[:])
    return nc


def _split_multi_waits(nc, max_waits=1):
    """This walrus build encodes at most one sync-wait per CTRL instruction;
    hoist extra waits into single-wait NoOps placed just before."""
    for f in nc.m.functions:
        for bb in f.blocks:
            insts = list(bb.instructions)
            out = []
            changed = False
            for ins in insts:
                si = ins.sync_info
                if si is not None and si.on_wait and len(si.on_wait) > max_waits:
                    waits = list(si.on_wait)
                    for w in waits[:-max_waits]:
                        out.append(mybir.InstNoOp(
                            name=nc.get_next_instruction_name(),
                            sync_info=mybir.SyncInfo(on_wait=[w], on_update=[]),
                            bass_nofuse=True,
                            engine=ins.engine,
                        ))
                    si.on_wait = waits[-max_waits:]
                    ins.sync_info = si
                    changed = True
                out.append(ins)
            if changed:
                bb.instructions = out


def build_nc(repeat=1, **kw):
    nc = bass.Bass("TRN2", target_bir_lowering=False, debug=False,
                   num_devices=N_CORES)
    _build_kernel_body(nc, repeat=repeat, **kw)
    _split_multi_waits(nc)
    # populate .instr bytes for extended InstISA subclasses (the DVE
    # tensor_tensor_reduce) -- without this the NEFF compiler fails with
    # "ISA wrong length"
    from concourse.library_overlay import lower_extended_insts
    lower_extended_insts(nc)
    return nc


def kernel(x):
    """x: [8, 64, 32, 128, 128] f32 -> att [8, 64, 1, 1, 1] f32."""
    from concourse.bass_utils import run_bass_kernel_spmd

    x = np.ascontiguousarray(np.asarray(x, dtype=np.float32))
    assert x.shape == (B, C, D, H, W)
    nc = build_nc()
    in_maps = [{"x": x[i].reshape(C, S)} for i in range(N_CORES)]
    res = run_bass_kernel_spmd(nc, in_maps, core_ids=list(range(N_CORES)))
    att = np.stack([res.results[i]["att"].reshape(C) for i in range(N_CORES)])
    return att.reshape(B, C, 1, 1, 1).astype(np.float32)
